# revision 3
# baseline (speedup 1.0000x reference)
"""Trainium2 Bass kernel for nn_DiscreteStateTransition (NRI-style GNN message passing).

Reference computation (per batch b, time t):
  inputs[o]   = concat(x[b,o,t,:56], forward_probs[b,o,t,:8])          # [8, 64]
  pre_msg[e]  = concat(inputs[recv(e)], inputs[send(e)])               # [56, 128]
  h1          = relu(pre_msg @ W1 + b1)                                # [56, 512]
  msg         = relu(h1 @ W2 + b2)                                     # [56, 512]
  agg[o]      = sum over edges e with recv(e)==o of msg[e]             # [8, 512]
  out[o]      = concat(inputs[o], agg[o]) @ Wn + bn                    # [8, 64]

Key structure exploited here: pre_msg @ W1 = W1a^T inp[recv] + W1b^T inp[send],
so layer 1 is computed per NODE (8 columns) instead of per EDGE (56 columns):
  U[o] = inp[o] @ W1a + b1,  V[o] = inp[o] @ W1b    (PE, node columns only)
  h1[r,s] = relu(U[r] + V[s])                       (DVE broadcast adds + max)
This cuts the layer-1 PE work 3.5x. All matmul operands are bf16 (1 cycle/row
on the PE, vs 4 for fp32), with fp32 PSUM accumulation.

Performance notes (v2):
- The PE clock is HAM-gated: a cold kernel runs matmuls at 1.2 GHz, and any
  ~3.4us window of mostly-idle PE re-throttles it. Dummy 64-col matmuls (on a
  memset scratch, into a scratch PSUM slot) fill every PE stall during the
  ramp so the whole L2 phase runs at 2.4 GHz.
- b1 is folded into the U eviction (Act bias is free), so the h1 relu pass is
  an immediate max(h1, 0) on the DVE, which reaches the 4x tensor-scalar mode.
- The ramp critical path is the DVE FIFO: V(0) evictions -> adds(0, piece0) ->
  relu -> first L2 sweep; every later chunk's UV matmuls ride inside the
  previous chunk's L2 stream.
- In the last chunk the head PSUM accumulation is interleaved with the per-f2
  aggregation trees (own PSUM pool), so only one small matmul, one eviction
  and one DMA remain after the last L2 sweep.

Sharding: data-parallel over (B=4) x (T-halves=2) -> 8 cores. Each core owns one
(b, t-half) slice: [8 objects, 256 timesteps]. Weights replicated; the host
pre-packs all weights into the on-chip layouts (bf16, blocked) and pre-transposes
inputs to feature-major, so the device does no layout work at all.

The output head produces out^T ([K*K, obj, time]) directly from PSUM (Wn output
dims land on partitions); the host unshards with a final numpy transpose.
"""

import numpy as np
import ml_dtypes

import concourse.bacc as bacc
import concourse.mybir as mybir
import concourse.tile as tile
from concourse.bass_utils import run_bass_kernel_spmd

F32 = mybir.dt.float32
BF16 = mybir.dt.bfloat16
MM_DT = mybir.dt.bfloat16  # matmul operand dtype (kept as module attr for test.py)

# Problem constants (hardcoded per the harness contract).
B, O, T = 4, 8, 512
D = 64            # node feature size (56 + 8)
E = 56            # directed edges = O*(O-1)
H = 512           # msg hidden/out size
KK = 64           # K*K output features
TC = 256          # timesteps per core
TB = 64           # timesteps per chunk
NCHUNK = TC // TB # 4
CE = E * TB       # edge columns per chunk (3584)
NN = O * TB       # node columns per chunk (512)

NPBF = ml_dtypes.bfloat16

N_PREWARM = 64    # dummy 64-col matmuls to lift the HAM clock gate at start


def build_nc(mm_dt=MM_DT, repeat=1, l2share=None):
    """Build the per-core Bass program (same program on all 8 cores)."""
    nc = bacc.Bacc("TRN2", target_bir_lowering=False, debug=False)

    xt = nc.dram_tensor("xt", [D, O, TC], BF16, kind="ExternalInput").ap()
    w1 = nc.dram_tensor("w1", [D, 8, 128], BF16, kind="ExternalInput").ap()
    w2 = nc.dram_tensor("w2", [128, 4, 4, 128], BF16, kind="ExternalInput").ap()
    wnin = nc.dram_tensor("wnin", [D, KK], BF16, kind="ExternalInput").ap()
    wnagg = nc.dram_tensor("wnagg", [128, 4, KK], BF16, kind="ExternalInput").ap()
    b1 = nc.dram_tensor("b1", [128, 4], F32, kind="ExternalInput").ap()
    b2 = nc.dram_tensor("b2", [128, 4], F32, kind="ExternalInput").ap()
    bn = nc.dram_tensor("bn", [KK, 1], F32, kind="ExternalInput").ap()
    outT = nc.dram_tensor("outT", [KK, O, TC], F32, kind="ExternalOutput").ap()

    AF = mybir.ActivationFunctionType
    ALU = mybir.AluOpType

    with tile.TileContext(nc) as tc:
        with (
            tc.tile_pool(name="const", bufs=1) as const,
            tc.tile_pool(name="uvp", bufs=4) as uvp,
            tc.tile_pool(name="h1p", bufs=4) as h1p,
            tc.tile_pool(name="msgp", bufs=5) as msgp,
            tc.tile_pool(name="aggp", bufs=2) as aggp,
            tc.tile_pool(name="treep", bufs=1) as treep,
            tc.tile_pool(name="netp", bufs=2) as netp,
            tc.tile_pool(name="uvps", bufs=2, space="PSUM") as uvps,
            tc.tile_pool(name="mpps", bufs=3, space="PSUM") as mpps,
        ):
            # ---- PE prewarm scratch: memset needs no inputs, so the dummy
            # matmul stream starts as soon as the engine preambles finish ----
            wscr = const.tile([128, 64], BF16)
            nc.gpsimd.memset(wscr[:], 0.0)

            _warm_ctr = [0]

            def prewarm(n):
                """n dummy 64-col matmuls into the (otherwise idle during the
                ramp) mpps PSUM ring; fills PE stalls so the HAM stays open."""
                _warm_ctr[0] += 1
                ps = mpps.tile([64, 64], F32, name=f"warm{_warm_ctr[0]}",
                               tag="mp")
                for i in range(n):
                    nc.tensor.matmul(ps[:], wscr[:], wscr[:],
                                     start=True, stop=(i == n - 1))

            prewarm(N_PREWARM)

            # ---- constants / weights (host pre-packed, no on-chip reformat).
            # DMA *issue* costs ~0.6-0.9us per descriptor on the queue engine,
            # so the order below is the ramp critical path: w1 + input chunk 0
            # first (UV matmuls), then b1 (U evictions), then the rest. ----
            w1s = const.tile([D, 8 * 128], BF16)
            nc.sync.dma_start(w1s[:], w1.rearrange("p j m -> p (j m)"))
            w1v = w1s.rearrange("p (j m) -> p j m", j=8)
            inT = const.tile([D, O * TC], BF16)
            inTv = inT.rearrange("p (o t) -> p o t", o=O)
            for c in range(NCHUNK):
                nc.gpsimd.dma_start(inTv[:, :, c * TB:(c + 1) * TB],
                                    xt[:, :, c * TB:(c + 1) * TB])
            b1t = const.tile([128, 4], F32)
            nc.sync.dma_start(b1t[:], b1)
            w2s = const.tile([128, 16 * 128], BF16)
            nc.sync.dma_start(w2s[:], w2.rearrange("p k f m -> p (k f m)"))
            w2v = w2s.rearrange("p (k f m) -> p k f m", k=4, f=4)
            b2t = const.tile([128, 4], F32)
            nc.sync.dma_start(b2t[:], b2)
            wnis = const.tile([D, KK], BF16)
            nc.sync.dma_start(wnis[:], wnin)
            wnas = const.tile([128, 4 * KK], BF16)
            nc.sync.dma_start(wnas[:], wnagg.rearrange("p k m -> p (k m)"))
            wnav = wnas.rearrange("p (k m) -> p k m", k=4)
            bnt = const.tile([KK, 1], F32)
            nc.sync.dma_start(bnt[:], bn)

            # ---- per-chunk unit builders ----
            usb = {}
            vsb = {}
            h1t = {}
            msgs = {}
            aggs = {}

            def uv_unit(c, j, evict="act"):
                """One W1 block j (0-3: U f-blocks, 4-7: V f-blocks) for chunk c.

                U evictions fold in b1 (Act bias); V evictions are plain
                copies, so h1 = relu(U' + V) needs only a max(.,0) pass."""
                t0 = c * TB
                ps = uvps.tile([128, NN], F32, name="uvps", tag="uvps")
                nc.tensor.matmul(ps[:], w1v[:, j, :], inTv[:, :, t0:t0 + TB],
                                 start=True, stop=True)
                dst = usb[c] if j < 4 else vsb[c]
                f = j % 4
                if j < 4:
                    nc.scalar.activation(dst[:, f, :], ps[:], AF.Identity,
                                         bias=b1t[:, f:f + 1])
                elif evict == "act":
                    nc.scalar.copy(dst[:, f, :], ps[:])
                else:
                    nc.vector.tensor_copy(dst[:, f, :], ps[:])

            def make_uv_tiles(c):
                usb[c] = uvp.tile([128, 4, NN], BF16, name=f"u{c}", tag="usb")
                vsb[c] = uvp.tile([128, 4, NN], BF16, name=f"v{c}", tag="vsb")
                h1t[c] = h1p.tile([128, 4, CE], BF16, name=f"h1_{c}", tag="h1")

            # L2 sweeps: three 1024-col + one 512-col (512-col matmul blocks
            # minimize the per-matmul weight-load tax; 512 is the ISA cap)
            SWEEPS = ((0, 1024), (1024, 1024), (2048, 1024), (3072, 512))
            NSW = len(SWEEPS)
            # recv-groups (448 cols each) covering each sweep's columns
            PIECE_RS = ((0, 1, 2), (3, 4), (5, 6), (7,))

            def adds(c, rs):
                """h1[r, s-slots] = U'[r] + V[send] for the given recv groups."""
                uvw = usb[c].rearrange("p f (o t) -> p f o t", o=O)
                vvw = vsb[c].rearrange("p f (o t) -> p f o t", o=O)
                h1v = h1t[c].rearrange("p f (r s t) -> p f r s t", r=O, s=O - 1)
                with nc.allow_low_precision(reason="bf16 h1 assembly"):
                    for r in rs:
                        ub = uvw[:, :, r, :].unsqueeze(2)
                        if r > 0:
                            nc.vector.tensor_add(
                                h1v[:, :, r, 0:r, :], vvw[:, :, 0:r, :],
                                ub.broadcast_to([128, 4, r, TB]))
                        if r < O - 1:
                            nc.vector.tensor_add(
                                h1v[:, :, r, r:O - 1, :], vvw[:, :, r + 1:O, :],
                                ub.broadcast_to([128, 4, O - 1 - r, TB]))

            def relu(c, piece=None):
                """In-place h1 = max(h1, 0) (DVE 4x tensor-scalar mode)."""
                h1v = h1t[c]
                if piece is None:
                    sl = h1v.rearrange("p f e -> p (f e)")
                else:
                    c0, w = SWEEPS[piece]
                    sl = h1v[:, :, c0:c0 + w]
                nc.vector.tensor_scalar(
                    out=sl, in0=sl, scalar1=0.0, scalar2=None, op0=ALU.max)

            def prep(c, piece=None):
                """DVE h1-prep, sweep-aligned pieces (pipelines with the L2)."""
                if piece is None:
                    adds(c, range(O))
                    relu(c)
                else:
                    adds(c, PIECE_RS[piece])
                    relu(c, piece)

            def l2_sweep(c, piece, f2):
                """One sweep x one f2: 4k accumulation over 512-col blocks."""
                c0, w = SWEEPS[piece]
                ncb = w // 512
                h1v = h1t[c]
                mp = mpps.tile([128, 2, 512], F32, name="mp", tag="mp")
                for k in range(4):
                    for cb in range(ncb):
                        nc.tensor.matmul(
                            mp[:, cb, :],
                            w2v[:, k, f2, :],
                            h1v[:, k, c0 + cb * 512:c0 + (cb + 1) * 512],
                            start=(k == 0), stop=(k == 3))
                dst = msgs[(c, f2)][:, c0:c0 + w]
                if ncb == 2:
                    dst = dst.rearrange("p (c x) -> p c x", c=2)
                    nc.scalar.activation(dst, mp[:, :, :], AF.Relu,
                                         bias=b2t[:, f2:f2 + 1])
                else:
                    nc.scalar.activation(dst, mp[:, 0, :], AF.Relu,
                                         bias=b2t[:, f2:f2 + 1])

            def trees(c, f2):
                """agg[r] = sum_s msg[r, s] via bf16 add tree on the DVE."""
                mg = msgs[(c, f2)].rearrange("p (r s t) -> p r s t", r=O, s=O - 1)
                with nc.allow_low_precision(reason="bf16 aggregation"):
                    t1 = treep.tile([128, O, 3, TB], BF16, name="t1", tag="t1v")
                    nc.vector.tensor_add(t1[:], mg[:, :, 0:6:2, :], mg[:, :, 1:7:2, :])
                    t2 = treep.tile([128, O, TB], BF16, name="t2", tag="t2v")
                    nc.vector.tensor_add(t2[:], t1[:, :, 0, :], t1[:, :, 1, :])
                    t3 = treep.tile([128, O, TB], BF16, name="t3", tag="t3v")
                    nc.vector.tensor_add(t3[:], t2[:], t1[:, :, 2, :])
                    agv = aggs[c].rearrange("p f (o t) -> p f o t", o=O)
                    nc.vector.tensor_add(agv[:, f2, :, :], t3[:], mg[:, :, 6, :])

            def trees_split(c, f2):
                """trees(c, f2) in r-pair groups aligned to sweep pieces, so
                the aggregation overlaps the f2's sweeps and only the last
                r-pair remains after the final eviction (short drain)."""
                mg = msgs[(c, f2)].rearrange("p (r s t) -> p r s t", r=O, s=O - 1)
                agv = aggs[c].rearrange("p f (o t) -> p f o t", o=O)
                with nc.allow_low_precision(reason="bf16 aggregation"):
                    for g in range(4):
                        r0 = 2 * g
                        t1 = treep.tile([128, 2, 3, TB], BF16, name="t1s",
                                        tag="t1v")
                        nc.vector.tensor_add(t1[:], mg[:, r0:r0 + 2, 0:6:2, :],
                                             mg[:, r0:r0 + 2, 1:7:2, :])
                        t2 = treep.tile([128, 2, TB], BF16, name="t2s",
                                        tag="t2v")
                        nc.vector.tensor_add(t2[:], t1[:, :, 0, :],
                                             t1[:, :, 1, :])
                        t3 = treep.tile([128, 2, TB], BF16, name="t3s",
                                        tag="t3v")
                        nc.vector.tensor_add(t3[:], t2[:], t1[:, :, 2, :])
                        nc.vector.tensor_add(agv[:, f2, r0:r0 + 2, :], t3[:],
                                             mg[:, r0:r0 + 2, 6, :])

            def head_start(c):
                """Start the head PSUM accumulation: input part only."""
                t0 = c * TB
                ps = mpps.tile([KK, NN], F32, name="np", tag="mp")
                nc.tensor.matmul(ps[:], wnis[:], inTv[:, :, t0:t0 + TB],
                                 start=True, stop=False)
                return ps

            def head_k(c, ps, k):
                """Accumulate agg f-block k into the head PSUM."""
                agv = aggs[c].rearrange("p f (o t) -> p f o t", o=O)
                nc.tensor.matmul(ps[:], wnav[:, k, :], agv[:, k, :, :],
                                 start=False, stop=(k == 3))

            def head_finish(c, ps):
                """Evict the head PSUM and DMA the chunk out."""
                t0 = c * TB
                net = netp.tile([KK, O, TB], F32, name="net", tag="net")
                nc.scalar.activation(net[:], ps.rearrange("p (o t) -> p o t", o=O),
                                     AF.Identity, bias=bnt[:])
                nc.sync.dma_start(outT[:, :, t0:t0 + TB], net[:])

            def head(c):
                ps = head_start(c)
                for k in range(4):
                    head_k(c, ps, k)
                head_finish(c, ps)


            def make_msg_tiles(c):
                msgs.update({(c, f2): msgp.tile([128, CE], BF16,
                                                name=f"m{c}_{f2}", tag="msg")
                             for f2 in range(4)})
                aggs[c] = aggp.tile([128, 4, NN], BF16, name=f"a{c}", tag="agg")

            # warm the activation-function table before it's needed
            scratch = const.tile([128, 1], F32)
            nc.vector.memset(scratch[:], 0.0)
            nc.scalar.activation(scratch[:], scratch[:], AF.Relu)

            # ---- static tiles: one SBUF buffer per chunk for U/V/h1 (pool
            # bufs match the tile count, so slots are stable across loop
            # iterations and the h1(0) rebuild never aliases h1(3)) ----
            for c in range(NCHUNK):
                make_uv_tiles(c)

            # ---- pre-loop ramp: only chunk 0's UV + h1. The DVE FIFO is the
            # ramp critical path: V(0) casts -> adds(0, p0) -> relu -> first
            # L2 sweep; UV(1) rides inside the first chunk like every other
            # chunk. Dummies fill all PE stalls (HAM stays at 2.4 GHz). ----
            for f in range(4):
                uv_unit(0, f)
                uv_unit(0, 4 + f, evict="dve")
                prewarm(10)
            prewarm(34)
            prewarm(52)
            for piece in range(NSW):
                prep(0, piece)

            def iteration():
                make_msg_tiles(0)

                # ---- steady state: UV(c+1) rides inside L2(c)'s first two
                # sweeps (so its evictions finish mid-chunk and prep(c+1) can
                # run on DVE while L2(c) still streams), trees(c) follow ----
                for c in range(NCHUNK):
                    last = c == NCHUNK - 1
                    if last:
                        # f2-major ordering: msg(c, f2) completes after f2's
                        # four sweeps, so trees/head drain overlaps the L2.
                        # head(c-1) rides after the first f2's sweeps instead
                        # of after the whole chunk (its trees finished long
                        # ago). For a repeat build, the h1(0) rebuild for the
                        # next iteration fills the DVE slots between trees.
                        for f2 in range(4):
                            for piece in range(NSW):
                                l2_sweep(c, piece, f2)
                            if f2 == 0:
                                head(c - 1)
                            if f2 == 3:
                                trees_split(c, f2)
                            else:
                                trees(c, f2)
                            if repeat > 1:
                                prep(0, f2)
                        head(c)
                    else:
                        for s in range(NSW):
                            if s == 0:
                                make_msg_tiles(c + 1)
                            for f2 in range(4):
                                l2_sweep(c, s, f2)
                            if s < 2:
                                for j in range(4 * s, 4 * s + 4):
                                    uv_unit(c + 1, j,
                                            evict=("act" if j < 4 else "dve"))
                        for piece in range(NSW):
                            prep(c + 1, piece)
                        for f2 in range(4):
                            trees(c, f2)
                    if c >= 1 and not last:
                        head(c - 1)

            # first pass runs outside any hardware loop; further passes are
            # unrolled UNROLL-deep inside a For_i so the per-iteration engine
            # rendezvous/branch cost (~5us) amortizes
            iteration()
            if repeat > 1:
                rem = repeat - 1
                unroll = 4 if rem % 4 == 0 else (2 if rem % 2 == 0 else 1)
                with tc.For_i(0, rem // unroll, 1,
                              hint_engines=(mybir.EngineType.PE,
                                            mybir.EngineType.DVE,
                                            mybir.EngineType.Activation)):
                    for _ in range(unroll):
                        iteration()

    nc.compile()
    return nc


_NC_CACHE = {}


def _get_nc():
    key = (MM_DT, 1)
    if key not in _NC_CACHE:
        _NC_CACHE[key] = build_nc(MM_DT, 1)
    return _NC_CACHE[key]


def shard_inputs(x, forward_probs, **_):
    """Per-core inputs: feature-major bf16 concat(x, fp) slabs."""
    xcat = np.concatenate(
        [np.asarray(x, dtype=np.float32),
         np.asarray(forward_probs, dtype=np.float32)], axis=-1)
    xbf = xcat.astype(NPBF)                       # [B, O, T, 64]
    in_maps = []
    for c in range(8):
        b, th = c // 2, c % 2
        slab = xbf[b, :, th * TC:(th + 1) * TC, :]        # [O, TC, 64]
        in_maps.append({"xt": np.ascontiguousarray(slab.transpose(2, 0, 1))})
    return in_maps


def prep_weights(W1, b1, W2, b2, Wn, bn, **_):
    """Host-side packing of all weights into on-chip layouts."""
    W1 = np.asarray(W1, dtype=np.float32)
    W2 = np.asarray(W2, dtype=np.float32)
    Wn = np.asarray(Wn, dtype=np.float32)
    w1p = np.stack([W1[0:64, j * 128:(j + 1) * 128] for j in range(4)]
                   + [W1[64:128, j * 128:(j + 1) * 128] for j in range(4)],
                   axis=1)                                  # [64, 8, 128]
    w2p = W2.reshape(4, 128, 4, 128).transpose(1, 0, 2, 3)  # [128, k, f2, 128]
    wnagg = Wn[64:].reshape(4, 128, KK).transpose(1, 0, 2)  # [128, k, 64]
    return {
        "w1": np.ascontiguousarray(w1p).astype(NPBF),
        "w2": np.ascontiguousarray(w2p).astype(NPBF),
        "wnin": np.ascontiguousarray(Wn[0:64]).astype(NPBF),
        "wnagg": np.ascontiguousarray(wnagg).astype(NPBF),
        "b1": np.ascontiguousarray(
            np.asarray(b1, dtype=np.float32).reshape(4, 128).T),
        "b2": np.ascontiguousarray(
            np.asarray(b2, dtype=np.float32).reshape(4, 128).T),
        "bn": np.ascontiguousarray(
            np.asarray(bn, dtype=np.float32).reshape(KK, 1)),
    }


def kernel(y, x, hidden_states, forward_probs, edge_est, edge_gt,
           W1, b1, W2, b2, Wn, bn, edge2node):
    nc = _get_nc()
    weights = prep_weights(W1, b1, W2, b2, Wn, bn)
    in_maps = [dict(m, **weights) for m in shard_inputs(x, forward_probs)]
    res = run_bass_kernel_spmd(nc, in_maps, list(range(8)))
    full = np.empty((B, O, T, KK), dtype=np.float32)
    for c in range(8):
        b, th = c // 2, c % 2
        # outT is [KK, O, TC]; undo the feature-major layout on the host
        full[b, :, th * TC:(th + 1) * TC, :] = \
            np.asarray(res.results[c]["outT"]).transpose(1, 2, 0)
    return full.reshape(B, O, T, 8, 8)


# revision 5
# speedup vs baseline: 1.1110x; 1.1110x over previous
"""Trainium2 Bass kernel for nn_DiscreteStateTransition (NRI-style GNN message passing).

Reference computation (per batch b, time t):
  inputs[o]   = concat(x[b,o,t,:56], forward_probs[b,o,t,:8])          # [8, 64]
  pre_msg[e]  = concat(inputs[recv(e)], inputs[send(e)])               # [56, 128]
  h1          = relu(pre_msg @ W1 + b1)                                # [56, 512]
  msg         = relu(h1 @ W2 + b2)                                     # [56, 512]
  agg[o]      = sum over edges e with recv(e)==o of msg[e]             # [8, 512]
  out[o]      = concat(inputs[o], agg[o]) @ Wn + bn                    # [8, 64]

Key structure exploited here: pre_msg @ W1 = W1a^T inp[recv] + W1b^T inp[send],
so layer 1 is computed per NODE (8 columns) instead of per EDGE (56 columns):
  U[o] = inp[o] @ W1a + b1,  V[o] = inp[o] @ W1b    (PE, node columns only)
  h1[r,s] = relu(U[r] + V[s])                       (DVE broadcast adds + max)
This cuts the layer-1 PE work 3.5x. All matmul operands are bf16 (1 cycle/row
on the PE, vs 4 for fp32), with fp32 PSUM accumulation.

Performance notes (v2):
- The PE clock is HAM-gated: a cold kernel runs matmuls at 1.2 GHz, and any
  ~3.4us window of mostly-idle PE re-throttles it. Dummy 64-col matmuls (on a
  memset scratch, into a scratch PSUM slot) fill every PE stall during the
  ramp so the whole L2 phase runs at 2.4 GHz.
- b1 is folded into the U eviction (Act bias is free), so the h1 relu pass is
  an immediate max(h1, 0) on the DVE, which reaches the 4x tensor-scalar mode.
- The ramp critical path is the DVE FIFO: V(0) evictions -> adds(0, piece0) ->
  relu -> first L2 sweep; every later chunk's UV matmuls ride inside the
  previous chunk's L2 stream.
- In the last chunk the head PSUM accumulation is interleaved with the per-f2
  aggregation trees (own PSUM pool), so only one small matmul, one eviction
  and one DMA remain after the last L2 sweep.

Sharding: data-parallel over (B=4) x (T-halves=2) -> 8 cores. Each core owns one
(b, t-half) slice: [8 objects, 256 timesteps]. Weights replicated; the host
pre-packs all weights into the on-chip layouts (bf16, blocked) and pre-transposes
inputs to feature-major, so the device does no layout work at all.

The output head produces out^T ([K*K, obj, time]) directly from PSUM (Wn output
dims land on partitions); the host unshards with a final numpy transpose.
"""

import numpy as np
import ml_dtypes

import concourse.bacc as bacc
import concourse.mybir as mybir
import concourse.tile as tile
from concourse.bass_utils import run_bass_kernel_spmd

F32 = mybir.dt.float32
BF16 = mybir.dt.bfloat16
MM_DT = mybir.dt.bfloat16  # matmul operand dtype (kept as module attr for test.py)

# Problem constants (hardcoded per the harness contract).
B, O, T = 4, 8, 512
D = 64            # node feature size (56 + 8)
E = 56            # directed edges = O*(O-1)
H = 512           # msg hidden/out size
KK = 64           # K*K output features
TC = 256          # timesteps per core
TB = 64           # timesteps per chunk
NCHUNK = TC // TB # 4
CE = E * TB       # edge columns per chunk (3584)
NN = O * TB       # node columns per chunk (512)

NPBF = ml_dtypes.bfloat16

N_PREWARM = 64    # dummy 64-col matmuls to lift the HAM clock gate at start


def build_nc(mm_dt=MM_DT, repeat=1, l2share=None):
    """Build the per-core Bass program (same program on all 8 cores)."""
    nc = bacc.Bacc("TRN2", target_bir_lowering=False, debug=False)

    xt = nc.dram_tensor("xt", [D, O, TC], BF16, kind="ExternalInput").ap()
    w1 = nc.dram_tensor("w1", [D, 8, 128], BF16, kind="ExternalInput").ap()
    w2 = nc.dram_tensor("w2", [128, 4, 4, 128], BF16, kind="ExternalInput").ap()
    wnin = nc.dram_tensor("wnin", [D, KK], BF16, kind="ExternalInput").ap()
    wnagg = nc.dram_tensor("wnagg", [128, 4, KK], BF16, kind="ExternalInput").ap()
    b1 = nc.dram_tensor("b1", [128, 4], F32, kind="ExternalInput").ap()
    b2 = nc.dram_tensor("b2", [128, 4], F32, kind="ExternalInput").ap()
    bn = nc.dram_tensor("bn", [KK, 1], F32, kind="ExternalInput").ap()
    outT = nc.dram_tensor("outT", [KK, O, TC], F32, kind="ExternalOutput").ap()

    AF = mybir.ActivationFunctionType
    ALU = mybir.AluOpType

    with tile.TileContext(nc) as tc:
        with (
            tc.tile_pool(name="const", bufs=1) as const,
            tc.tile_pool(name="uvp", bufs=4) as uvp,
            tc.tile_pool(name="h1p", bufs=4) as h1p,
            tc.tile_pool(name="msgp", bufs=5) as msgp,
            tc.tile_pool(name="aggp", bufs=2) as aggp,
            tc.tile_pool(name="treep", bufs=1) as treep,
            tc.tile_pool(name="netp", bufs=2) as netp,
            tc.tile_pool(name="uvps", bufs=2, space="PSUM") as uvps,
            tc.tile_pool(name="mpps", bufs=3, space="PSUM") as mpps,
        ):
            # ---- PE prewarm scratch: memset needs no inputs, so the dummy
            # matmul stream starts as soon as the engine preambles finish ----
            wscr = const.tile([128, 64], BF16)
            nc.gpsimd.memset(wscr[:], 0.0)

            _warm_ctr = [0]

            def prewarm(n):
                """n dummy 64-col matmuls into the (otherwise idle during the
                ramp) mpps PSUM ring; fills PE stalls so the HAM stays open."""
                _warm_ctr[0] += 1
                ps = mpps.tile([64, 64], F32, name=f"warm{_warm_ctr[0]}",
                               tag="mp")
                for i in range(n):
                    nc.tensor.matmul(ps[:], wscr[:], wscr[:],
                                     start=True, stop=(i == n - 1))

            prewarm(N_PREWARM)

            # ---- constants / weights (host pre-packed, no on-chip reformat).
            # DMA *issue* costs ~0.6-0.9us per descriptor on the queue engine,
            # so the order below is the ramp critical path: w1 + input chunk 0
            # first (UV matmuls), then b1 (U evictions), then the rest. ----
            w1s = const.tile([D, 8 * 128], BF16)
            nc.sync.dma_start(w1s[:], w1.rearrange("p j m -> p (j m)"))
            w1v = w1s.rearrange("p (j m) -> p j m", j=8)
            inT = const.tile([D, O * TC], BF16)
            inTv = inT.rearrange("p (o t) -> p o t", o=O)
            for c in range(NCHUNK):
                nc.gpsimd.dma_start(inTv[:, :, c * TB:(c + 1) * TB],
                                    xt[:, :, c * TB:(c + 1) * TB])
            b1t = const.tile([128, 4], F32)
            nc.sync.dma_start(b1t[:], b1)
            w2s = const.tile([128, 16 * 128], BF16)
            nc.sync.dma_start(w2s[:], w2.rearrange("p k f m -> p (k f m)"))
            w2v = w2s.rearrange("p (k f m) -> p k f m", k=4, f=4)
            b2t = const.tile([128, 4], F32)
            nc.sync.dma_start(b2t[:], b2)
            wnis = const.tile([D, KK], BF16)
            nc.sync.dma_start(wnis[:], wnin)
            wnas = const.tile([128, 4 * KK], BF16)
            nc.sync.dma_start(wnas[:], wnagg.rearrange("p k m -> p (k m)"))
            wnav = wnas.rearrange("p (k m) -> p k m", k=4)
            bnt = const.tile([KK, 1], F32)
            nc.sync.dma_start(bnt[:], bn)

            # ---- per-chunk unit builders ----
            usb = {}
            vsb = {}
            h1t = {}
            msgs = {}
            aggs = {}

            def uv_unit(c, j, evict="act"):
                """One W1 block j (0-3: U f-blocks, 4-7: V f-blocks) for chunk c.

                U evictions fold in b1 (Act bias); V evictions are plain
                copies, so h1 = relu(U' + V) needs only a max(.,0) pass."""
                t0 = c * TB
                ps = uvps.tile([128, NN], F32, name="uvps", tag="uvps")
                nc.tensor.matmul(ps[:], w1v[:, j, :], inTv[:, :, t0:t0 + TB],
                                 start=True, stop=True)
                dst = usb[c] if j < 4 else vsb[c]
                f = j % 4
                if j < 4:
                    nc.scalar.activation(dst[:, f, :], ps[:], AF.Identity,
                                         bias=b1t[:, f:f + 1])
                elif evict == "act":
                    nc.scalar.copy(dst[:, f, :], ps[:])
                else:
                    nc.vector.tensor_copy(dst[:, f, :], ps[:])

            def make_uv_tiles(c):
                usb[c] = uvp.tile([128, 4, NN], BF16, name=f"u{c}", tag="usb")
                vsb[c] = uvp.tile([128, 4, NN], BF16, name=f"v{c}", tag="vsb")
                h1t[c] = h1p.tile([128, 4, CE], BF16, name=f"h1_{c}", tag="h1")

            # L2 sweeps: three 1024-col + one 512-col (512-col matmul blocks
            # minimize the per-matmul weight-load tax; 512 is the ISA cap)
            SWEEPS = ((0, 1024), (1024, 1024), (2048, 1024), (3072, 512))
            NSW = len(SWEEPS)
            # recv-groups (448 cols each) covering each sweep's columns
            PIECE_RS = ((0, 1, 2), (3, 4), (5, 6), (7,))

            def adds(c, rs):
                """h1[r, s-slots] = U'[r] + V[send] for the given recv groups."""
                uvw = usb[c].rearrange("p f (o t) -> p f o t", o=O)
                vvw = vsb[c].rearrange("p f (o t) -> p f o t", o=O)
                h1v = h1t[c].rearrange("p f (r s t) -> p f r s t", r=O, s=O - 1)
                with nc.allow_low_precision(reason="bf16 h1 assembly"):
                    for r in rs:
                        ub = uvw[:, :, r, :].unsqueeze(2)
                        if r > 0:
                            nc.vector.tensor_add(
                                h1v[:, :, r, 0:r, :], vvw[:, :, 0:r, :],
                                ub.broadcast_to([128, 4, r, TB]))
                        if r < O - 1:
                            nc.vector.tensor_add(
                                h1v[:, :, r, r:O - 1, :], vvw[:, :, r + 1:O, :],
                                ub.broadcast_to([128, 4, O - 1 - r, TB]))

            def relu(c, piece=None):
                """In-place h1 = max(h1, 0) (DVE 4x tensor-scalar mode)."""
                h1v = h1t[c]
                if piece is None:
                    sl = h1v.rearrange("p f e -> p (f e)")
                else:
                    c0, w = SWEEPS[piece]
                    sl = h1v[:, :, c0:c0 + w]
                nc.vector.tensor_scalar(
                    out=sl, in0=sl, scalar1=0.0, scalar2=None, op0=ALU.max)

            def prep(c, piece=None):
                """DVE h1-prep, sweep-aligned pieces (pipelines with the L2)."""
                if piece is None:
                    adds(c, range(O))
                    relu(c)
                else:
                    adds(c, PIECE_RS[piece])
                    relu(c, piece)

            def l2_sweep(c, piece, f2):
                """One sweep x one f2: 4k accumulation over 512-col blocks."""
                c0, w = SWEEPS[piece]
                ncb = w // 512
                h1v = h1t[c]
                mp = mpps.tile([128, 2, 512], F32, name="mp", tag="mp")
                for k in range(4):
                    for cb in range(ncb):
                        nc.tensor.matmul(
                            mp[:, cb, :],
                            w2v[:, k, f2, :],
                            h1v[:, k, c0 + cb * 512:c0 + (cb + 1) * 512],
                            start=(k == 0), stop=(k == 3))
                dst = msgs[(c, f2)][:, c0:c0 + w]
                if ncb == 2:
                    dst = dst.rearrange("p (c x) -> p c x", c=2)
                    nc.scalar.activation(dst, mp[:, :, :], AF.Relu,
                                         bias=b2t[:, f2:f2 + 1])
                else:
                    nc.scalar.activation(dst, mp[:, 0, :], AF.Relu,
                                         bias=b2t[:, f2:f2 + 1])

            def trees(c, f2):
                """agg[r] = sum_s msg[r, s] via bf16 add tree on the DVE."""
                mg = msgs[(c, f2)].rearrange("p (r s t) -> p r s t", r=O, s=O - 1)
                with nc.allow_low_precision(reason="bf16 aggregation"):
                    t1 = treep.tile([128, O, 3, TB], BF16, name="t1", tag="t1v")
                    nc.vector.tensor_add(t1[:], mg[:, :, 0:6:2, :], mg[:, :, 1:7:2, :])
                    t2 = treep.tile([128, O, TB], BF16, name="t2", tag="t2v")
                    nc.vector.tensor_add(t2[:], t1[:, :, 0, :], t1[:, :, 1, :])
                    t3 = treep.tile([128, O, TB], BF16, name="t3", tag="t3v")
                    nc.vector.tensor_add(t3[:], t2[:], t1[:, :, 2, :])
                    agv = aggs[c].rearrange("p f (o t) -> p f o t", o=O)
                    nc.vector.tensor_add(agv[:, f2, :, :], t3[:], mg[:, :, 6, :])

            def trees_split(c, f2):
                """trees(c, f2) in r-pair groups aligned to sweep pieces, so
                the aggregation overlaps the f2's sweeps and only the last
                r-pair remains after the final eviction (short drain)."""
                mg = msgs[(c, f2)].rearrange("p (r s t) -> p r s t", r=O, s=O - 1)
                agv = aggs[c].rearrange("p f (o t) -> p f o t", o=O)
                with nc.allow_low_precision(reason="bf16 aggregation"):
                    for g in range(4):
                        r0 = 2 * g
                        t1 = treep.tile([128, 2, 3, TB], BF16, name="t1s",
                                        tag="t1v")
                        nc.vector.tensor_add(t1[:], mg[:, r0:r0 + 2, 0:6:2, :],
                                             mg[:, r0:r0 + 2, 1:7:2, :])
                        t2 = treep.tile([128, 2, TB], BF16, name="t2s",
                                        tag="t2v")
                        nc.vector.tensor_add(t2[:], t1[:, :, 0, :],
                                             t1[:, :, 1, :])
                        t3 = treep.tile([128, 2, TB], BF16, name="t3s",
                                        tag="t3v")
                        nc.vector.tensor_add(t3[:], t2[:], t1[:, :, 2, :])
                        nc.vector.tensor_add(agv[:, f2, r0:r0 + 2, :], t3[:],
                                             mg[:, r0:r0 + 2, 6, :])

            def head_start(c):
                """Start the head PSUM accumulation: input part only."""
                t0 = c * TB
                ps = mpps.tile([KK, NN], F32, name="np", tag="mp")
                nc.tensor.matmul(ps[:], wnis[:], inTv[:, :, t0:t0 + TB],
                                 start=True, stop=False)
                return ps

            def head_k(c, ps, k):
                """Accumulate agg f-block k into the head PSUM."""
                agv = aggs[c].rearrange("p f (o t) -> p f o t", o=O)
                nc.tensor.matmul(ps[:], wnav[:, k, :], agv[:, k, :, :],
                                 start=False, stop=(k == 3))

            def head_finish(c, ps):
                """Evict the head PSUM and DMA the chunk out."""
                t0 = c * TB
                net = netp.tile([KK, O, TB], F32, name="net", tag="net")
                nc.scalar.activation(net[:], ps.rearrange("p (o t) -> p o t", o=O),
                                     AF.Identity, bias=bnt[:])
                nc.sync.dma_start(outT[:, :, t0:t0 + TB], net[:])

            def head(c):
                ps = head_start(c)
                for k in range(4):
                    head_k(c, ps, k)
                head_finish(c, ps)


            def make_msg_tiles(c):
                msgs.update({(c, f2): msgp.tile([128, CE], BF16,
                                                name=f"m{c}_{f2}", tag="msg")
                             for f2 in range(4)})
                aggs[c] = aggp.tile([128, 4, NN], BF16, name=f"a{c}", tag="agg")

            # warm the activation-function table before it's needed
            scratch = const.tile([128, 1], F32)
            nc.vector.memset(scratch[:], 0.0)
            nc.scalar.activation(scratch[:], scratch[:], AF.Relu)

            # ---- static tiles: one SBUF buffer per chunk for U/V/h1 (pool
            # bufs match the tile count, so slots are stable across loop
            # iterations and the h1(0) rebuild never aliases h1(3)) ----
            for c in range(NCHUNK):
                make_uv_tiles(c)

            # ---- pre-loop ramp: only chunk 0's UV + h1. The DVE FIFO is the
            # ramp critical path: V(0) casts -> adds(0, p0) -> relu -> first
            # L2 sweep; UV(1) rides inside the first chunk like every other
            # chunk. Dummies fill all PE stalls (HAM stays at 2.4 GHz). ----
            for f in range(4):
                uv_unit(0, f)
                uv_unit(0, 4 + f, evict="dve")
                prewarm(10)
            prewarm(34)
            prewarm(52)
            prewarm(18)
            for piece in range(NSW):
                prep(0, piece)

            def iteration():
                make_msg_tiles(0)

                # ---- steady state: UV(c+1) rides inside L2(c)'s first two
                # sweeps (so its evictions finish mid-chunk and prep(c+1) can
                # run on DVE while L2(c) still streams), trees(c) follow ----
                for c in range(NCHUNK):
                    last = c == NCHUNK - 1
                    if last:
                        # f2-major ordering: msg(c, f2) completes after f2's
                        # four sweeps, so trees/head drain overlaps the L2.
                        # head(c-1) rides after the first f2's sweeps instead
                        # of after the whole chunk (its trees finished long
                        # ago). For a repeat build, the h1(0) rebuild for the
                        # next iteration fills the DVE slots between trees.
                        for f2 in range(4):
                            for piece in range(NSW):
                                l2_sweep(c, piece, f2)
                            if f2 == 0:
                                head(c - 1)
                            if f2 == 3:
                                trees_split(c, f2)
                            else:
                                trees(c, f2)
                            if repeat > 1:
                                prep(0, f2)
                        head(c)
                    else:
                        for s in range(NSW):
                            if s == 0:
                                make_msg_tiles(c + 1)
                            if s < 2:
                                # riding UV first: their evictions enter the
                                # Act/DVE FIFOs ahead of this sweep's msg
                                # evictions, so the uv PSUM ring never waits
                                # behind the eviction backlog
                                for j in range(4 * s, 4 * s + 4):
                                    uv_unit(c + 1, j,
                                            evict=("act" if j < 4 else "dve"))
                            for f2 in range(4):
                                l2_sweep(c, s, f2)
                        for piece in range(NSW):
                            prep(c + 1, piece)
                        for f2 in range(4):
                            trees(c, f2)
                    if c >= 1 and not last:
                        head(c - 1)

            # first pass runs outside any hardware loop; further passes are
            # unrolled UNROLL-deep inside a For_i so the per-iteration engine
            # rendezvous/branch cost (~5us) amortizes
            iteration()
            if repeat > 1:
                rem = repeat - 1
                unroll = (8 if rem % 8 == 0 else 4 if rem % 4 == 0 else
                          2 if rem % 2 == 0 else 1)
                with tc.For_i(0, rem // unroll, 1,
                              hint_engines=(mybir.EngineType.PE,
                                            mybir.EngineType.DVE,
                                            mybir.EngineType.Activation)):
                    for _ in range(unroll):
                        iteration()

    nc.compile()
    return nc


_NC_CACHE = {}


def _get_nc():
    key = (MM_DT, 1)
    if key not in _NC_CACHE:
        _NC_CACHE[key] = build_nc(MM_DT, 1)
    return _NC_CACHE[key]


def shard_inputs(x, forward_probs, **_):
    """Per-core inputs: feature-major bf16 concat(x, fp) slabs."""
    xcat = np.concatenate(
        [np.asarray(x, dtype=np.float32),
         np.asarray(forward_probs, dtype=np.float32)], axis=-1)
    xbf = xcat.astype(NPBF)                       # [B, O, T, 64]
    in_maps = []
    for c in range(8):
        b, th = c // 2, c % 2
        slab = xbf[b, :, th * TC:(th + 1) * TC, :]        # [O, TC, 64]
        in_maps.append({"xt": np.ascontiguousarray(slab.transpose(2, 0, 1))})
    return in_maps


def prep_weights(W1, b1, W2, b2, Wn, bn, **_):
    """Host-side packing of all weights into on-chip layouts."""
    W1 = np.asarray(W1, dtype=np.float32)
    W2 = np.asarray(W2, dtype=np.float32)
    Wn = np.asarray(Wn, dtype=np.float32)
    w1p = np.stack([W1[0:64, j * 128:(j + 1) * 128] for j in range(4)]
                   + [W1[64:128, j * 128:(j + 1) * 128] for j in range(4)],
                   axis=1)                                  # [64, 8, 128]
    w2p = W2.reshape(4, 128, 4, 128).transpose(1, 0, 2, 3)  # [128, k, f2, 128]
    wnagg = Wn[64:].reshape(4, 128, KK).transpose(1, 0, 2)  # [128, k, 64]
    return {
        "w1": np.ascontiguousarray(w1p).astype(NPBF),
        "w2": np.ascontiguousarray(w2p).astype(NPBF),
        "wnin": np.ascontiguousarray(Wn[0:64]).astype(NPBF),
        "wnagg": np.ascontiguousarray(wnagg).astype(NPBF),
        "b1": np.ascontiguousarray(
            np.asarray(b1, dtype=np.float32).reshape(4, 128).T),
        "b2": np.ascontiguousarray(
            np.asarray(b2, dtype=np.float32).reshape(4, 128).T),
        "bn": np.ascontiguousarray(
            np.asarray(bn, dtype=np.float32).reshape(KK, 1)),
    }


def kernel(y, x, hidden_states, forward_probs, edge_est, edge_gt,
           W1, b1, W2, b2, Wn, bn, edge2node):
    nc = _get_nc()
    weights = prep_weights(W1, b1, W2, b2, Wn, bn)
    in_maps = [dict(m, **weights) for m in shard_inputs(x, forward_probs)]
    res = run_bass_kernel_spmd(nc, in_maps, list(range(8)))
    full = np.empty((B, O, T, KK), dtype=np.float32)
    for c in range(8):
        b, th = c // 2, c % 2
        # outT is [KK, O, TC]; undo the feature-major layout on the host
        full[b, :, th * TC:(th + 1) * TC, :] = \
            np.asarray(res.results[c]["outT"]).transpose(1, 2, 0)
    return full.reshape(B, O, T, 8, 8)
